# revision 23
# baseline (speedup 1.0000x reference)
"""Trainium2 Bass kernel for nn_BiLingual (dual embedding gather + cAddTanh pool).

Computes, for two embedding tables:
    out[t, b, :] = sum_{j=0}^{S-2} tanh(W_t[idx_t[b, j]] + W_t[idx_t[b, j+1]])

Sharding: data-parallel over batch. Each of the 8 cores handles 8 batch rows
for BOTH tables; tables are replicated (converted to bf16 on the host).

Per-core device layout (v3): partition p owns positions [16p, 16p+16) of the
sequence; gathered rows land column-major (slot s -> partition s%128, column
s//128, position = 16*(s%128) + s//128 + 16*c0).  Consecutive positions of a
pair then sit on the SAME partition in ADJACENT columns, so the pairwise add
is a legal free-axis-offset DVE op.  The per-partition boundary pair
(16p+15, 16p+16) gets its right element from one extra 128-idx gather.

Pipeline per sequence row:
  1. dma_gather (gpsimd SWDGE, bf16 tables, 512B/descriptor):
       stream A: cols c=0..3   (512 idxs + 16 biased-0 guards -> junk col 4)
       stream B: cols c=4..7   (512 idxs + 16 guards -> junk col 9)
       stream C: cols c=8..11  (512 idxs + 16 guards -> junk col 14)
       stream D: cols c=12..15 + boundary rights pos 16p+16 (640 idxs ->
                 cols 15-19; final slot is masked and forced to biased-0 so
                 the ucode's trailing-negative trim never fires).
     num_idxs > 1024 hangs the SWDGE ucode (HW-bisected); 4 equal streams
     per sequence keep the 4 SWDGE queues' waves balanced.
     Streams round-robin over the 4 SWDGE queues so all four Q7 core pairs
     generate descriptors concurrently.  int16 index range via biasing:
     base = W[32768:], idx' = idx - 32768.
  2. DVE adds (free-axis column offsets, all partitions base-0):
       seven ops walking the 4-col blocks, skipping the junk cols; the last
       covers pairs c12..c14 plus the boundary pair (valid p<127).
  3. ACT tanh A -> T (bf16), one [128, 4096] instruction per sequence.
  4. PE masked ones-matmul reduces T into a [16, 256] PSUM accumulator
     (output partition = table*8 + local_row); mask ty0 = all partitions
     (in-partition pairs), ty1 = p<127 (boundary column).
"""
import os

import numpy as np

from concourse import bacc, mybir
import concourse.tile as tile
from concourse.bass_utils import run_bass_kernel_spmd

P = 128
B, S, V, D = 64, 2048, 50000, 256
N_CORES = 8
B_LOC = B // N_CORES        # 8 batch rows per core
CPP = 16                    # positions per partition
NCOL = 16                   # result pair-columns per sequence (15 main + 1 boundary)
NROW = 2 * B_LOC            # 16 (table, local row) pairs per core
SPLIT = 32768
N_QUEUES = int(os.environ.get("KQUEUES", "4"))

# per-row gather streams: (first pos-col, n idxs incl guards, dst col, dst ncol)
# num_idxs > 1024 hangs the SWDGE ucode (HW-bisected); 4 equal streams per
# sequence keep all 4 SWDGE queues busy with balanced ~4.8us waves.
STREAMS = [
    (0, 4 * P + 16, 0, 5),      # c0-3   + guards -> junk col 4
    (4, 4 * P + 16, 5, 5),      # c4-7   + guards -> junk col 9
    (8, 4 * P + 16, 10, 5),     # c8-11  + guards -> junk col 14
    (12, 4 * P + 16, 15, 5),    # c12-15 + guards -> junk col 19
]
IDX_COLS = 48               # idx tile column pitch per stream (>= 640/16, 32B-aligned)
N_SLOTS = NROW * len(STREAMS)

_last_results = None        # set by _run for test harness introspection


def _build_red_masks():
    # red[:, (row16*2 + ty)*16 : +16]: column row16 holds mask_ty, rest 0.
    # ty=0: all partitions valid (in-partition pairs); ty=1: p < 127 (boundary).
    red = np.zeros((P, NROW * 2 * 16), dtype=np.float32)
    masks = [
        np.ones(P, dtype=np.float32),
        (np.arange(P) < 127).astype(np.float32),
    ]
    for row16 in range(NROW):
        for ty in range(2):
            red[:, (row16 * 2 + ty) * 16 + row16] = masks[ty]
    return red


def _split_multi_waits(nc, max_waits=1):
    """Walrus rejects instructions carrying too many sync waits; hoist excess
    waits onto same-engine NOPs inserted just before the instruction (engine
    program order makes this equivalent)."""
    for bb in nc.main_func.blocks:
        idx = 0
        while idx < len(bb.instructions):
            ins = bb.instructions[idx]
            si = ins.sync_info
            if si is not None and si.on_wait and len(si.on_wait) > max_waits:
                waits = list(si.on_wait)
                extra, keep = waits[:-max_waits], waits[-max_waits:]
                for w0 in range(0, len(extra), max_waits):
                    nop = mybir.InstNoOp(
                        name=nc.get_next_instruction_name(), ins=[], outs=[]
                    )
                    nop.engine = ins.engine
                    nop.sync_info = mybir.SyncInfo(
                        on_wait=extra[w0 : w0 + max_waits], on_update=[]
                    )
                    nc.register_instruction(nop)
                    bb.instructions.insert(idx, nop)
                    idx += 1
                si.on_wait = keep
            idx += 1


def _build_program():
    nc = bacc.Bacc(None, target_bir_lowering=False, num_swdge_queues=N_QUEUES)
    bf16 = mybir.dt.bfloat16
    Wp = nc.declare_dram_parameter("W_pri", [V, D], bf16, isOutput=False)
    Ws = nc.declare_dram_parameter("W_sec", [V, D], bf16, isOutput=False)
    idxA = nc.declare_dram_parameter(
        "idxA", [P, N_SLOTS * IDX_COLS], mybir.dt.int16, isOutput=False
    )
    red = nc.declare_dram_parameter(
        "red", [P, NROW * 2 * 16], mybir.dt.float32, isOutput=False
    )
    s1T = nc.declare_dram_parameter("s1T", [P, P], bf16, isOutput=False)
    out = nc.declare_dram_parameter("out", [NROW, D], mybir.dt.float32, isOutput=True)

    with tile.TileContext(nc) as tc:
        # one shared num_idxs register per distinct stream size: avoids a
        # per-gather RegisterMove on the gpsimd engine (each costs ~0.4us of
        # hoisted semaphore-wait time at startup)
        sizes = sorted({s[1] for s in STREAMS})
        regs = {}
        import contextlib
        reg_stack = contextlib.ExitStack()
        for sz in sizes:
            r = reg_stack.enter_context(nc.gpsimd.register(f"nidx_{sz}"))
            nc.gpsimd.reg_mov(r, sz)
            regs[sz] = r
        with (
            tc.tile_pool(name="const", bufs=1) as const,
            tc.tile_pool(name="ebuf", bufs=8) as ebuf,
            tc.tile_pool(name="abuf", bufs=4) as abuf,
            tc.tile_pool(name="tbuf", bufs=4) as tbuf,
            tc.tile_pool(name="psR", bufs=1, space="PSUM") as psR,
            tc.tile_pool(name="psB", bufs=2, space="PSUM") as psB,
            tc.tile_pool(name="osb", bufs=1) as osb,
        ):
            iA = const.tile([P, N_SLOTS * IDX_COLS], mybir.dt.int16)
            nc.sync.dma_start(out=iA[:], in_=idxA[:])
            s1t = const.tile([P, P], bf16)
            nc.sync.dma_start(out=s1t[:], in_=s1T[:])
            red_f32 = const.tile([P, NROW * 2 * 16], mybir.dt.float32)
            nc.sync.dma_start(out=red_f32[:], in_=red[:])
            red_t = const.tile([P, NROW * 2 * 16], bf16)
            nc.vector.tensor_copy(out=red_t[:], in_=red_f32[:])

            acc = psR.tile([NROW, D], mybir.dt.float32, space="PSUM")
            n_red = NROW * NCOL
            red_i = 0

            for t, W in enumerate((Wp, Ws)):
                for r in range(B_LOC):
                    row16 = t * B_LOC + r
                    e = ebuf.tile([P, 20, D], bf16)
                    for k, (c0, nidx, d0, ncol) in enumerate(STREAMS):
                        slot = row16 * len(STREAMS) + k
                        nc.gpsimd.dma_gather(
                            out_ap=e[:, d0 : d0 + ncol, :],
                            in_ap=W[SPLIT:, :],
                            idxs_ap=iA[
                                :, slot * IDX_COLS : slot * IDX_COLS + nidx // 16
                            ],
                            num_idxs=nidx,
                            num_idxs_reg=regs[nidx],
                            elem_size=D,
                            queue_num=k % N_QUEUES,
                        )
                    if os.environ.get("KSTAGE") == "gather":
                        last_e = e
                        continue
                    # boundary rights = e[p+1, c0] via PE partition shift; the
                    # boundary pair (16p+15, 16p+16) then adds on DVE from
                    # PSUM + e's c15 column (no dedicated boundary gather).
                    ps_bd = psB.tile([P, D], mybir.dt.float32, space="PSUM")
                    nc.tensor.matmul(
                        out=ps_bd[:],
                        lhsT=s1t[:],
                        rhs=e[:, 0, :],
                        start=True,
                        stop=True,
                    )
                    a = abuf.tile([P, NCOL, D], bf16)
                    nc.vector.tensor_add(
                        out=a[:, 0:3, :], in0=e[:, 0:3, :], in1=e[:, 1:4, :]
                    )
                    nc.vector.tensor_add(
                        out=a[:, 3:4, :], in0=e[:, 3:4, :], in1=e[:, 5:6, :]
                    )
                    nc.vector.tensor_add(
                        out=a[:, 4:7, :], in0=e[:, 5:8, :], in1=e[:, 6:9, :]
                    )
                    nc.vector.tensor_add(
                        out=a[:, 7:8, :], in0=e[:, 8:9, :], in1=e[:, 10:11, :]
                    )
                    nc.vector.tensor_add(
                        out=a[:, 8:11, :], in0=e[:, 10:13, :], in1=e[:, 11:14, :]
                    )
                    nc.vector.tensor_add(
                        out=a[:, 11:12, :], in0=e[:, 13:14, :], in1=e[:, 15:16, :]
                    )
                    nc.vector.tensor_add(
                        out=a[:, 12:15, :], in0=e[:, 15:18, :], in1=e[:, 16:19, :]
                    )
                    nc.vector.tensor_add(
                        out=a[:, 15:16, :], in0=ps_bd[:, :], in1=e[:, 18:19, :]
                    )
                    tt = tbuf.tile([P, NCOL, D], bf16)
                    for h in range(2):
                        nc.scalar.activation(
                            tt[:, h * 8 : (h + 1) * 8, :].rearrange(
                                "p g d -> p (g d)"
                            ),
                            a[:, h * 8 : (h + 1) * 8, :].rearrange(
                                "p g d -> p (g d)"
                            ),
                            mybir.ActivationFunctionType.Tanh,
                        )
                    for g in range(NCOL):
                        ty = 1 if g == NCOL - 1 else 0
                        nc.tensor.matmul(
                            out=acc[:],
                            lhsT=red_t[
                                :, (row16 * 2 + ty) * 16 : (row16 * 2 + ty + 1) * 16
                            ],
                            rhs=tt[:, g, :],
                            start=(red_i == 0),
                            stop=(red_i == n_red - 1),
                        )
                        red_i += 1

            res_sb = osb.tile([NROW, D], mybir.dt.float32)
            if os.environ.get("KSTAGE") == "gather":
                nc.vector.tensor_copy(out=res_sb[:], in_=last_e[0:NROW, 0, :])
            else:
                nc.scalar.copy(out=res_sb[:], in_=acc[:])
            nc.sync.dma_start(out=out[:], in_=res_sb[:])

        reg_stack.close()

    nc.compile()
    _split_multi_waits(nc)
    return nc


def _host_prep(inputs_pri, inputs_sec, W_pri, W_sec):
    import ml_dtypes

    ip = np.asarray(inputs_pri).astype(np.int64, copy=False)
    is_ = np.asarray(inputs_sec).astype(np.int64, copy=False)
    wp = np.ascontiguousarray(np.asarray(W_pri, dtype=np.float32)).astype(
        ml_dtypes.bfloat16
    )
    ws = np.ascontiguousarray(np.asarray(W_sec, dtype=np.float32)).astype(
        ml_dtypes.bfloat16
    )
    red = _build_red_masks()
    s1T = np.zeros((P, P), dtype=np.float32)
    s1T[np.arange(1, P), np.arange(P - 1)] = 1.0  # out[p] = in[p+1]
    s1T = s1T.astype(ml_dtypes.bfloat16)

    p_ar = np.arange(P)
    in_maps = []
    for k in range(N_CORES):
        idxA = np.zeros((P, N_SLOTS * IDX_COLS), dtype=np.int16)
        for t, idx in enumerate((ip, is_)):
            for r in range(B_LOC):
                row16 = t * B_LOC + r
                seq = idx[k * B_LOC + r]  # [S]
                for s, (c0, nidx, d0, ncol) in enumerate(STREAMS):
                    # slot s -> partition s%128, col c0 + s//128,
                    # position 16*(s%128) + (c0 + s//128)
                    pos = (CPP * p_ar[None, :] + c0 + np.arange(4)[:, None]).reshape(
                        -1
                    )  # [512] in slot order (col-major)
                    stream = (seq[pos] - SPLIT).astype(np.int16)
                    stream = np.concatenate(
                        [stream, np.zeros(nidx - 4 * P, np.int16)]
                    )
                    slot = row16 * len(STREAMS) + s
                    if N_QUEUES == 4:
                        # queue s's Q7 core pair reads only partitions
                        # [32s, 32s+32); don't replicate to the other stripes
                        wrapped = np.tile(stream.reshape(-1, 16).T, (2, 1))
                        idxA[
                            32 * s : 32 * s + 32,
                            slot * IDX_COLS : slot * IDX_COLS + nidx // 16,
                        ] = wrapped
                    else:
                        wrapped = np.tile(stream.reshape(-1, 16).T, (8, 1))
                        idxA[:, slot * IDX_COLS : slot * IDX_COLS + nidx // 16] = (
                            wrapped
                        )
        in_maps.append(
            {
                "W_pri": wp,
                "W_sec": ws,
                "idxA": idxA,
                "red": red,
                "s1T": s1T,
            }
        )
    return in_maps


def _run(inputs_pri, inputs_sec, W_pri, W_sec, trace=False):
    global _last_results
    nc = _build_program()
    in_maps = _host_prep(inputs_pri, inputs_sec, W_pri, W_sec)
    res = run_bass_kernel_spmd(nc, in_maps, list(range(N_CORES)), trace=trace)
    _last_results = res
    out = np.empty((2, B, D), dtype=np.float32)
    for k in range(N_CORES):
        o = res.results[k]["out"]  # [16, 256]
        out[0, k * B_LOC : (k + 1) * B_LOC] = o[:B_LOC]
        out[1, k * B_LOC : (k + 1) * B_LOC] = o[B_LOC:]
    return out


def kernel(inputs_pri, inputs_sec, W_pri, W_sec):
    trace = bool(int(os.environ.get("KERNEL_TRACE", "0")))
    return _run(inputs_pri, inputs_sec, W_pri, W_sec, trace=trace)


# revision 24
# speedup vs baseline: 1.0150x; 1.0150x over previous
"""Trainium2 Bass kernel for nn_BiLingual (dual embedding gather + cAddTanh pool).

Computes, for two embedding tables:
    out[t, b, :] = sum_{j=0}^{S-2} tanh(W_t[idx_t[b, j]] + W_t[idx_t[b, j+1]])

Sharding: data-parallel over batch. Each of the 8 cores handles 8 batch rows
for BOTH tables; tables are replicated (converted to bf16 on the host, which
halves gather bytes and keeps every PE matmul single-pass with fast weight
loads; measured rel err ~2e-3 vs the 2e-2 gate).

Per-core device layout: partition p owns positions [16p, 16p+16) of the
sequence; gathered rows land column-major (slot s -> partition s%128, column
s//128, position = 16*(s%128) + s//128 + 16*c0).  Consecutive positions of a
pair then sit on the SAME partition in ADJACENT columns, so the pairwise add
is a legal free-axis-offset DVE op (cross-partition operands are rejected by
the BIR verifier).  The per-partition boundary pair (16p+15, 16p+16) gets its
right element from duplicated rows in the fourth gather stream.

Pipeline per sequence row (16 rows per core):
  1. dma_gather (gpsimd SWDGE, bf16 tables, 512B/descriptor):
       stream A: cols c=0..3   (512 idxs + 16 biased-0 guards -> junk col 4)
       stream B: cols c=4..7   (512 idxs + 16 guards -> junk col 9)
       stream C: cols c=8..11  (512 idxs + 16 guards -> junk col 14)
       stream D: cols c=12..15 + boundary rights pos 16p+16 (640 idxs ->
                 cols 15-19; final slot is masked and forced to biased-0 so
                 the ucode's trailing-negative trim never fires).
     HW-characterized constraints baked in here:
       - num_idxs > 1024 hangs the SWDGE ucode (bisected: 912 ok, 1040 hang).
       - stream k runs on SWDGE queue k (num_swdge_queues=4): each queue is
         served by its own Q7 core pair (the ucode dispatches the pair with
         cpu_id/2 == queue_num), so the four streams' descriptor generation
         runs 4-way parallel.  This is the single biggest win vs the 1-queue
         baseline: descriptor generation is the kernel's bottleneck
         (~9.3 ns/idx per core pair; gathers alone account for ~121 of the
         ~128 us end-to-end).
       - tile's DMA semaphores are locked to one queue each and recycled by
         emission order, so queue must be a pure function of emission
         position (rotating queue assignments regresses 4-25%).
       - the idx tile column pitch must keep slots 32B-aligned (the Q7
         vector pops misread at 16B alignment and the wild addresses wedge
         the device).
       - only the owning queue's core pair (partitions [32q, 32q+32)) reads
         a slot's idxs, so each slot is packed into just that stripe.
     int16 idx range via biasing: base = W[32768:], idx' = idx - 32768
     (the Q7 address math is signed; HW-verified).
  2. DVE adds (free-axis column offsets, all partitions base-0): seven ops
     walking the 4-col blocks, skipping the junk cols; the last covers pairs
     c12..c14 plus the boundary pair (valid p<127 in the boundary column).
  3. ACT tanh A -> T (bf16), two [128, 2048] instructions per sequence so
     the PE reduce of the first half overlaps the second half's tanh.
  4. PE masked ones-matmul reduces T into a [16, 256] PSUM accumulator
     (output partition = table*8 + local_row); mask ty0 = all partitions
     (in-partition pairs), ty1 = p<127 (boundary column).
"""
import os

import numpy as np

from concourse import bacc, mybir
import concourse.tile as tile
from concourse.bass_utils import run_bass_kernel_spmd

P = 128
B, S, V, D = 64, 2048, 50000, 256
N_CORES = 8
B_LOC = B // N_CORES        # 8 batch rows per core
CPP = 16                    # positions per partition
NCOL = 16                   # result pair-columns per sequence (15 main + 1 boundary)
NROW = 2 * B_LOC            # 16 (table, local row) pairs per core
SPLIT = 32768
N_QUEUES = int(os.environ.get("KQUEUES", "4"))

# per-row gather streams: (first pos-col, n idxs incl guards, dst col, dst ncol)
STREAMS = [
    (0, 4 * P + 16, 0, 5),      # c0-3  + guards -> junk col 4
    (4, 4 * P + 16, 5, 5),      # c4-7  + guards -> junk col 9
    (8, 4 * P + 16, 10, 5),     # c8-11 + guards -> junk col 14
    (-1, 5 * P, 15, 5),         # c12-15 + boundary rights (no guards)
]
IDX_COLS = 48               # idx tile column pitch per stream (>= 640/16, 32B-aligned)
N_SLOTS = NROW * len(STREAMS)

_last_results = None        # set by _run for test harness introspection


def _build_red_masks():
    # red[:, (row16*2 + ty)*16 : +16]: column row16 holds mask_ty, rest 0.
    # ty=0: all partitions valid (in-partition pairs); ty=1: p < 127 (boundary).
    red = np.zeros((P, NROW * 2 * 16), dtype=np.float32)
    masks = [
        np.ones(P, dtype=np.float32),
        (np.arange(P) < 127).astype(np.float32),
    ]
    for row16 in range(NROW):
        for ty in range(2):
            red[:, (row16 * 2 + ty) * 16 + row16] = masks[ty]
    return red


def _split_multi_waits(nc, max_waits=1):
    """Walrus rejects instructions carrying too many sync waits; hoist excess
    waits onto same-engine NOPs inserted just before the instruction (engine
    program order makes this equivalent)."""
    for bb in nc.main_func.blocks:
        idx = 0
        while idx < len(bb.instructions):
            ins = bb.instructions[idx]
            si = ins.sync_info
            if si is not None and si.on_wait and len(si.on_wait) > max_waits:
                waits = list(si.on_wait)
                extra, keep = waits[:-max_waits], waits[-max_waits:]
                for w0 in range(0, len(extra), max_waits):
                    nop = mybir.InstNoOp(
                        name=nc.get_next_instruction_name(), ins=[], outs=[]
                    )
                    nop.engine = ins.engine
                    nop.sync_info = mybir.SyncInfo(
                        on_wait=extra[w0 : w0 + max_waits], on_update=[]
                    )
                    nc.register_instruction(nop)
                    bb.instructions.insert(idx, nop)
                    idx += 1
                si.on_wait = keep
            idx += 1


def _build_program():
    nc = bacc.Bacc(None, target_bir_lowering=False, num_swdge_queues=N_QUEUES)
    bf16 = mybir.dt.bfloat16
    Wp = nc.declare_dram_parameter("W_pri", [V, D], bf16, isOutput=False)
    Ws = nc.declare_dram_parameter("W_sec", [V, D], bf16, isOutput=False)
    idxA = nc.declare_dram_parameter(
        "idxA", [P, N_SLOTS * IDX_COLS], mybir.dt.int16, isOutput=False
    )
    red = nc.declare_dram_parameter(
        "red", [P, NROW * 2 * 16], mybir.dt.float32, isOutput=False
    )
    out = nc.declare_dram_parameter("out", [NROW, D], mybir.dt.float32, isOutput=True)

    with tile.TileContext(nc) as tc:
        with (
            tc.tile_pool(name="const", bufs=1) as const,
            tc.tile_pool(name="ebuf", bufs=6) as ebuf,
            tc.tile_pool(name="abuf", bufs=4) as abuf,
            tc.tile_pool(name="tbuf", bufs=4) as tbuf,
            tc.tile_pool(name="psR", bufs=1, space="PSUM") as psR,
            tc.tile_pool(name="osb", bufs=1) as osb,
        ):
            red_f32 = const.tile([P, NROW * 2 * 16], mybir.dt.float32)
            nc.sync.dma_start(out=red_f32[:], in_=red[:])
            red_t = const.tile([P, NROW * 2 * 16], bf16)
            nc.vector.tensor_copy(out=red_t[:], in_=red_f32[:])
            iA = const.tile([P, N_SLOTS * IDX_COLS], mybir.dt.int16)
            nc.sync.dma_start(out=iA[:], in_=idxA[:])

            acc = psR.tile([NROW, D], mybir.dt.float32, space="PSUM")
            n_red = NROW * NCOL
            red_i = 0

            for t, W in enumerate((Wp, Ws)):
                for r in range(B_LOC):
                    row16 = t * B_LOC + r
                    e = ebuf.tile([P, 20, D], bf16)
                    for k, (c0, nidx, d0, ncol) in enumerate(STREAMS):
                        slot = row16 * len(STREAMS) + k
                        nc.gpsimd.dma_gather(
                            out_ap=e[:, d0 : d0 + ncol, :],
                            in_ap=W[SPLIT:, :],
                            idxs_ap=iA[
                                :, slot * IDX_COLS : slot * IDX_COLS + nidx // 16
                            ],
                            num_idxs=nidx,
                            num_idxs_reg=nidx,
                            elem_size=D,
                            queue_num=k % N_QUEUES,
                        )
                    a = abuf.tile([P, NCOL, D], bf16)
                    nc.vector.tensor_add(
                        out=a[:, 0:3, :], in0=e[:, 0:3, :], in1=e[:, 1:4, :]
                    )
                    nc.vector.tensor_add(
                        out=a[:, 3:4, :], in0=e[:, 3:4, :], in1=e[:, 5:6, :]
                    )
                    nc.vector.tensor_add(
                        out=a[:, 4:7, :], in0=e[:, 5:8, :], in1=e[:, 6:9, :]
                    )
                    nc.vector.tensor_add(
                        out=a[:, 7:8, :], in0=e[:, 8:9, :], in1=e[:, 10:11, :]
                    )
                    nc.vector.tensor_add(
                        out=a[:, 8:11, :], in0=e[:, 10:13, :], in1=e[:, 11:14, :]
                    )
                    nc.vector.tensor_add(
                        out=a[:, 11:12, :], in0=e[:, 13:14, :], in1=e[:, 15:16, :]
                    )
                    nc.vector.tensor_add(
                        out=a[:, 12:16, :], in0=e[:, 15:19, :], in1=e[:, 16:20, :]
                    )
                    tt = tbuf.tile([P, NCOL, D], bf16)
                    for h in range(2):
                        nc.scalar.activation(
                            tt[:, h * 8 : (h + 1) * 8, :].rearrange(
                                "p g d -> p (g d)"
                            ),
                            a[:, h * 8 : (h + 1) * 8, :].rearrange(
                                "p g d -> p (g d)"
                            ),
                            mybir.ActivationFunctionType.Tanh,
                        )
                    for g in range(NCOL):
                        ty = 1 if g == NCOL - 1 else 0
                        nc.tensor.matmul(
                            out=acc[:],
                            lhsT=red_t[
                                :, (row16 * 2 + ty) * 16 : (row16 * 2 + ty + 1) * 16
                            ],
                            rhs=tt[:, g, :],
                            start=(red_i == 0),
                            stop=(red_i == n_red - 1),
                        )
                        red_i += 1

            res_sb = osb.tile([NROW, D], mybir.dt.float32)
            nc.scalar.copy(out=res_sb[:], in_=acc[:])
            nc.sync.dma_start(out=out[:], in_=res_sb[:])

    nc.compile()
    _split_multi_waits(nc)
    return nc


def _host_prep(inputs_pri, inputs_sec, W_pri, W_sec):
    import ml_dtypes

    ip = np.asarray(inputs_pri).astype(np.int64, copy=False)
    is_ = np.asarray(inputs_sec).astype(np.int64, copy=False)
    wp = np.ascontiguousarray(np.asarray(W_pri, dtype=np.float32)).astype(
        ml_dtypes.bfloat16
    )
    ws = np.ascontiguousarray(np.asarray(W_sec, dtype=np.float32)).astype(
        ml_dtypes.bfloat16
    )
    red = _build_red_masks()

    p_ar = np.arange(P)
    in_maps = []
    for k in range(N_CORES):
        idxA = np.zeros((P, N_SLOTS * IDX_COLS), dtype=np.int16)
        for t, idx in enumerate((ip, is_)):
            for r in range(B_LOC):
                row16 = t * B_LOC + r
                seq = idx[k * B_LOC + r]  # [S]
                for s, (c0, nidx, d0, ncol) in enumerate(STREAMS):
                    if c0 >= 0:
                        # slot s -> partition s%128, col c0 + s//128,
                        # position 16*(s%128) + (c0 + s//128)
                        pos = (
                            CPP * p_ar[None, :] + c0 + np.arange(4)[:, None]
                        ).reshape(-1)  # [512] in slot order (col-major)
                        stream = (seq[pos] - SPLIT).astype(np.int16)
                        stream = np.concatenate(
                            [stream, np.zeros(nidx - 4 * P, np.int16)]
                        )
                    else:
                        # cols c12-15, then boundary rights (pos 16p+16,
                        # clamped); final slot (p=127) is masked out of the
                        # reduce -> biased-0 so the trailing-negative trim
                        # never fires.
                        pos = np.concatenate(
                            [
                                (
                                    CPP * p_ar[None, :]
                                    + 12
                                    + np.arange(4)[:, None]
                                ).reshape(-1),
                                np.minimum(CPP * p_ar + CPP, S - 1),
                            ]
                        )
                        stream = (seq[pos] - SPLIT).astype(np.int16)
                        stream[5 * P - 1] = 0
                    slot = row16 * len(STREAMS) + s
                    if N_QUEUES == 4:
                        # queue s's Q7 core pair reads only partitions
                        # [32s, 32s+32); don't replicate to the other stripes
                        wrapped = np.tile(stream.reshape(-1, 16).T, (2, 1))
                        idxA[
                            32 * s : 32 * s + 32,
                            slot * IDX_COLS : slot * IDX_COLS + nidx // 16,
                        ] = wrapped
                    else:
                        wrapped = np.tile(stream.reshape(-1, 16).T, (8, 1))
                        idxA[:, slot * IDX_COLS : slot * IDX_COLS + nidx // 16] = (
                            wrapped
                        )
        in_maps.append(
            {
                "W_pri": wp,
                "W_sec": ws,
                "idxA": idxA,
                "red": red,
            }
        )
    return in_maps


def _run(inputs_pri, inputs_sec, W_pri, W_sec, trace=False):
    global _last_results
    nc = _build_program()
    in_maps = _host_prep(inputs_pri, inputs_sec, W_pri, W_sec)
    res = run_bass_kernel_spmd(nc, in_maps, list(range(N_CORES)), trace=trace)
    _last_results = res
    out = np.empty((2, B, D), dtype=np.float32)
    for k in range(N_CORES):
        o = res.results[k]["out"]  # [16, 256]
        out[0, k * B_LOC : (k + 1) * B_LOC] = o[:B_LOC]
        out[1, k * B_LOC : (k + 1) * B_LOC] = o[B_LOC:]
    return out


def kernel(inputs_pri, inputs_sec, W_pri, W_sec):
    trace = bool(int(os.environ.get("KERNEL_TRACE", "0")))
    return _run(inputs_pri, inputs_sec, W_pri, W_sec, trace=trace)
